# revision 13
# baseline (speedup 1.0000x reference)
"""Trainium2 Bass kernel for nn_CustomizeLSTMCell.

reference:
    pre = w_in_input @ s_in + w_out_input @ s_out + u_in_input @ h_in + u_out_input @ h_out
    g = sigmoid(pre)
    cell_state = g * last_c + g * g          # = g * (last_c + g)
    hidden_state = g * cell_state
    returns (cell_state, hidden_state)       # each [H, B] f32

Sharding: pure data parallel along the batch (column) axis B across 8
NeuronCores; the four [128,128] weights are replicated (pre-transposed and
concatenated on host so they land in one DMA and feed the PE as lhsT).

The kernel is HBM-bandwidth bound (5 big loads + 2 big stores, tiny
matmuls), so all large tensors move as bf16: host downcasts inputs
(rel err ~1e-2 vs the 2e-2 gate, verified against the f32 oracle),
the PE accumulates bf16 operands into f32 PSUM, and outputs are stored
bf16 and upcast on host. This halves HBM traffic: 56 -> 28 MiB/core.

Lessons from the trace of v1:
  - weight DMAs must go FIRST on the same HWDGE ring as the big loads:
    on the store ring their 256 B/partition descriptors round-robin
    against saturated load packets and take ~45 us to land, serializing
    every matmul behind them.
  - matmuls are emitted weight-stationary (LS w_k once per 2048-col
    block, accumulating into 4 PSUM banks) so the PE stream is dense;
    a chopped PE stream never leaves the 1.2 GHz mid p-state.

Device-side layout (all pack/unpack happens on host):
  - the four matmul operands + last_c are interleaved per tile into one
    flat DRAM band [128, 5*B_core], so each 2048-col tile is ONE
    2.62 MB load instead of 5 smaller ones.
  - per tile, cell/hidden outputs are written side by side in one SBUF
    tile and leave via ONE 1.05 MB store into a flat [128, 2*B_core]
    band.
"""

import sys
from contextlib import ExitStack

import numpy as np
import ml_dtypes

for _p in ("/opt/trn_rl_repo", "/opt/pypackages"):
    if _p not in sys.path:
        sys.path.append(_p)

import concourse.bass as bass
import concourse.tile as tile
from concourse import bacc, mybir
from concourse import bass_utils

H = 128
S = 128
B = 131072
N_CORES = 8
B_CORE = B // N_CORES  # 16384 columns per core

N_TILE = 2048          # columns per load tile == elementwise/store block
MM_FREE = 512          # matmul free dim = one PSUM bank of f32

F32 = mybir.dt.float32
BF16 = mybir.dt.bfloat16
NP_BF16 = ml_dtypes.bfloat16

MM_INPUTS = ("s_in", "s_out", "h_in", "h_out")  # packed, matmul operands
WEIGHTS = ("w_in_input", "w_out_input", "u_in_input", "u_out_input")
N_MM = len(MM_INPUTS)
N_OPS = N_MM + 1  # + last_c riding along in the packed tile


def tile_plan(b_core: int):
    """List of (col_offset, tile_cols). The final N_TILE columns taper off
    (1024, 512, 512) so the endgame load->compute->store chain after the
    very last load is shallow (the PE is HAM-cold by then)."""
    n_full = b_core // N_TILE - 1
    plan = [(i * N_TILE, N_TILE) for i in range(n_full)]
    base = n_full * N_TILE
    for tc in (N_TILE // 2, N_TILE // 4, N_TILE // 4):
        plan.append((base, tc))
        base += tc
    return plan


def pack_mm_inputs(arrs, b_core: int):
    """[5][128, b_core] f32 -> bf16 [128, 5*b_core]: each tile from
    tile_plan() is a contiguous [s_in|s_out|h_in|h_out|last_c] segment of
    width 5*tile_cols."""
    out = np.empty((H, N_OPS * b_core), dtype=NP_BF16)
    for off, tc in tile_plan(b_core):
        col = N_OPS * off
        for k, a in enumerate(arrs):
            out[:, col + k * tc : col + (k + 1) * tc] = a[:, off : off + tc]
    return out


def unpack_outputs(packed, b_core: int):
    """bf16 [128, 2*b_core] tile-major [c_t | h_t] -> (cell, hidden) f32."""
    c = np.empty((H, b_core), dtype=np.float32)
    h = np.empty((H, b_core), dtype=np.float32)
    for off, tc in tile_plan(b_core):
        seg = packed[:, 2 * off : 2 * off + 2 * tc]
        c[:, off : off + tc] = seg[:, :tc]
        h[:, off : off + tc] = seg[:, tc : 2 * tc]
    return c, h


def emit_lstm_tile(ctx: ExitStack, tc: tile.TileContext, io: dict, b_core: int):
    """Per-core body.

    - the packed weight load issues FIRST on the Sync HWDGE ring, then the
      tile loads (same ring, FIFO -> weights land before tile 0 finishes);
      stores go on the Scalar HWDGE ring
    - per tile: weight-stationary matmul sweep (for each of the 4 weights,
      LS once then one 512-col matmul per PSUM bank, accumulating), then
      ACT sigmoid PSUM -> bf16 SBUF per bank, then 3 DVE bf16 ops, then
      one packed c|h store whose issue is delayed by one tile so the
      Scalar engine never stalls waiting on DVE results.
    """
    nc = tc.nc

    wpool = ctx.enter_context(tc.tile_pool(name="weights", bufs=1))
    inpool = ctx.enter_context(tc.tile_pool(name="inp", bufs=5))
    work = ctx.enter_context(tc.tile_pool(name="work", bufs=5))
    psum = ctx.enter_context(tc.tile_pool(name="psum", bufs=8, space="PSUM"))

    w_cat = wpool.tile([S, N_MM * H], BF16, name="w_cat")
    nc.sync.dma_start(w_cat[:], io["w_cat"][:])
    wtiles = [w_cat[:, bass.ts(k, H)] for k in range(N_MM)]

    pending_store = None  # (ch_tile, col_offset)

    def flush_store():
        nonlocal pending_store
        if pending_store is not None:
            ch, off, tcols = pending_store
            nc.scalar.dma_start(
                io["out_packed"][:, 2 * off : 2 * off + 2 * tcols], ch[:]
            )
            pending_store = None

    for off, tcols in tile_plan(b_core):
        n_chunks = tcols // MM_FREE
        t_in = inpool.tile([S, N_OPS * tcols], BF16, name="t_in")
        nc.sync.dma_start(
            t_in[:],
            io["in_packed"][:, N_OPS * off : N_OPS * (off + tcols)],
        )
        # operand views inside the packed tile
        ops = [t_in[:, bass.ts(k, tcols)] for k in range(N_MM)]
        t_lc = t_in[:, bass.ts(N_MM, tcols)]

        # weight-stationary sweep: LS w_k once, accumulate into all banks
        pss = [psum.tile([H, MM_FREE], F32, name="ps") for _ in range(n_chunks)]
        for k in range(N_MM):
            for j in range(n_chunks):
                nc.tensor.matmul(
                    pss[j][:], wtiles[k], ops[k][:, bass.ts(j, MM_FREE)],
                    start=(k == 0), stop=(k == N_MM - 1),
                )


        flush_store()  # previous tile's c|h are ready; don't queue the
        # store issue behind this tile's sigmoids on the Scalar FIFO

        g = work.tile([H, tcols], BF16, name="g")
        for j in range(n_chunks):
            nc.scalar.activation(
                g[:, bass.ts(j, MM_FREE)], pss[j][:],
                mybir.ActivationFunctionType.Sigmoid,
            )

        # c = g * (last_c + g); h = g * c  -- all on DVE, back to back
        tmp = work.tile([H, tcols], BF16, name="tmp")
        nc.vector.tensor_add(tmp[:], g[:], t_lc[:])
        ch = work.tile([H, 2 * tcols], BF16, name="ch")
        nc.vector.tensor_mul(ch[:, 0:tcols], g[:], tmp[:])
        last = off + tcols == b_core
        if last:
            # final tile: ship the c half while DVE computes the h half
            nc.scalar.dma_start(
                io["out_packed"][:, 2 * off : 2 * off + tcols], ch[:, 0:tcols]
            )
        nc.vector.tensor_mul(ch[:, tcols : 2 * tcols], g[:], ch[:, 0:tcols])
        if last:
            nc.scalar.dma_start(
                io["out_packed"][:, 2 * off + tcols : 2 * off + 2 * tcols],
                ch[:, tcols : 2 * tcols],
            )
        else:
            pending_store = (ch, off, tcols)

    flush_store()


def build_model(b_core: int = B_CORE, n_cores: int = N_CORES):
    nc = bacc.Bacc(
        "TRN2",
        target_bir_lowering=False,
        debug=False,
        enable_asserts=False,
        num_devices=n_cores,
    )
    io = {}
    io["in_packed"] = nc.dram_tensor(
        "in_packed", [S, N_OPS * b_core], BF16, kind="ExternalInput"
    ).ap()
    io["w_cat"] = nc.dram_tensor(
        "w_cat", [S, N_MM * H], BF16, kind="ExternalInput"
    ).ap()
    io["out_packed"] = nc.dram_tensor(
        "out_packed", [H, 2 * b_core], BF16, kind="ExternalOutput"
    ).ap()

    with tile.TileContext(nc) as tc, ExitStack() as ctx:
        emit_lstm_tile(ctx, tc, io, b_core)
    nc.compile()
    return nc


_model_cache: dict = {}


def _get_model():
    if "nc" not in _model_cache:
        _model_cache["nc"] = build_model()
    return _model_cache["nc"]


def make_in_maps(inputs: dict, b_core: int = B_CORE, n_cores: int = N_CORES):
    w_cat = np.concatenate(
        [np.asarray(inputs[w], dtype=np.float32).T for w in WEIGHTS], axis=1
    ).astype(NP_BF16)
    big = {k: np.asarray(inputs[k], dtype=np.float32) for k in MM_INPUTS + ("last_c",)}
    in_maps = []
    for c in range(n_cores):
        sl = slice(c * b_core, (c + 1) * b_core)
        m = {
            "in_packed": pack_mm_inputs(
                [big[k][:, sl] for k in MM_INPUTS + ("last_c",)], b_core
            ),
            "w_cat": w_cat,
        }
        in_maps.append(m)
    return in_maps


def run_spmd(inputs: dict, trace: bool = False, **kwargs):
    nc = _get_model()
    in_maps = make_in_maps(inputs)
    res = bass_utils.run_bass_kernel_spmd(
        nc, in_maps, core_ids=list(range(N_CORES)), trace=trace, **kwargs
    )
    cells, hiddens = [], []
    for c in range(N_CORES):
        cell, hidden = unpack_outputs(res.results[c]["out_packed"], B_CORE)
        cells.append(cell)
        hiddens.append(hidden)
    return (
        np.concatenate(cells, axis=1),
        np.concatenate(hiddens, axis=1),
    ), res


def kernel(**inputs):
    outs, _ = run_spmd(inputs, trace=False)
    return outs


# revision 14
# speedup vs baseline: 1.0379x; 1.0379x over previous
"""Trainium2 Bass kernel for nn_CustomizeLSTMCell.

reference:
    pre = w_in_input @ s_in + w_out_input @ s_out + u_in_input @ h_in + u_out_input @ h_out
    g = sigmoid(pre)
    cell_state = g * last_c + g * g          # = g * (last_c + g)
    hidden_state = g * cell_state
    returns (cell_state, hidden_state)       # each [H, B] f32

Sharding: pure data parallel along the batch (column) axis B across 8
NeuronCores; the four [128,128] weights are replicated (pre-transposed and
concatenated on host so they land in one DMA and feed the PE as lhsT).

The kernel is HBM-bandwidth bound (5 big loads + 2 big stores, tiny
matmuls), so all large tensors move as bf16: host downcasts inputs
(rel err ~1e-2 vs the 2e-2 gate, verified against the f32 oracle),
the PE accumulates bf16 operands into f32 PSUM, and outputs are stored
bf16 and upcast on host. This halves HBM traffic: 56 -> 28 MiB/core.

Lessons from the trace of v1:
  - weight DMAs must go FIRST on the same HWDGE ring as the big loads:
    on the store ring their 256 B/partition descriptors round-robin
    against saturated load packets and take ~45 us to land, serializing
    every matmul behind them.
  - matmuls are emitted weight-stationary (LS w_k once per 2048-col
    block, accumulating into 4 PSUM banks) so the PE stream is dense;
    a chopped PE stream never leaves the 1.2 GHz mid p-state.

Device-side layout (all pack/unpack happens on host):
  - the four matmul operands + last_c are interleaved per tile into one
    flat DRAM band [128, 5*B_core], so each 2048-col tile is ONE
    2.62 MB load instead of 5 smaller ones.
  - per tile, cell/hidden outputs are written side by side in one SBUF
    tile and leave via ONE 1.05 MB store into a flat [128, 2*B_core]
    band.
"""

import sys
from contextlib import ExitStack

import numpy as np
import ml_dtypes

for _p in ("/opt/trn_rl_repo", "/opt/pypackages"):
    if _p not in sys.path:
        sys.path.append(_p)

import concourse.bass as bass
import concourse.tile as tile
from concourse import bacc, mybir
from concourse import bass_utils

H = 128
S = 128
B = 131072
N_CORES = 8
B_CORE = B // N_CORES  # 16384 columns per core

N_TILE = 2048          # columns per load tile == elementwise/store block
MM_FREE = 512          # matmul free dim = one PSUM bank of f32

F32 = mybir.dt.float32
BF16 = mybir.dt.bfloat16
NP_BF16 = ml_dtypes.bfloat16

MM_INPUTS = ("s_in", "s_out", "h_in", "h_out")  # packed, matmul operands
WEIGHTS = ("w_in_input", "w_out_input", "u_in_input", "u_out_input")
N_MM = len(MM_INPUTS)
N_OPS = N_MM + 1  # + last_c riding along in the packed tile


def tile_plan(b_core: int):
    """List of (col_offset, tile_cols). The final N_TILE columns taper off
    (1024, 512, 512) so the endgame load->compute->store chain after the
    very last load is shallow (the PE is HAM-cold by then)."""
    n_full = b_core // N_TILE - 1
    plan = [(i * N_TILE, N_TILE) for i in range(n_full)]
    base = n_full * N_TILE
    for tc in (N_TILE // 2, N_TILE // 4, N_TILE // 4):
        plan.append((base, tc))
        base += tc
    return plan


def pack_mm_inputs(arrs, b_core: int):
    """[5][128, b_core] f32 -> bf16 [128, 5*b_core]: each tile from
    tile_plan() is a contiguous [s_in|s_out|h_in|h_out|last_c] segment of
    width 5*tile_cols."""
    out = np.empty((H, N_OPS * b_core), dtype=NP_BF16)
    for off, tc in tile_plan(b_core):
        col = N_OPS * off
        for k, a in enumerate(arrs):
            out[:, col + k * tc : col + (k + 1) * tc] = a[:, off : off + tc]
    return out


def unpack_outputs(packed, b_core: int):
    """bf16 [128, 2*b_core] tile-major [c_t | h_t] -> (cell, hidden) f32."""
    c = np.empty((H, b_core), dtype=np.float32)
    h = np.empty((H, b_core), dtype=np.float32)
    for off, tc in tile_plan(b_core):
        seg = packed[:, 2 * off : 2 * off + 2 * tc]
        c[:, off : off + tc] = seg[:, :tc]
        h[:, off : off + tc] = seg[:, tc : 2 * tc]
    return c, h


def emit_lstm_tile(ctx: ExitStack, tc: tile.TileContext, io: dict, b_core: int):
    """Per-core body.

    - the packed weight load issues FIRST on the Sync HWDGE ring, then the
      tile loads (same ring, FIFO -> weights land before tile 0 finishes);
      stores go on the Scalar HWDGE ring
    - per tile: weight-stationary matmul sweep (for each of the 4 weights,
      LS once then one 512-col matmul per PSUM bank, accumulating), then
      ACT sigmoid PSUM -> bf16 SBUF per bank, then 3 DVE bf16 ops, then
      one packed c|h store whose issue is delayed by one tile so the
      Scalar engine never stalls waiting on DVE results.
    """
    nc = tc.nc

    wpool = ctx.enter_context(tc.tile_pool(name="weights", bufs=1))
    inpool = ctx.enter_context(tc.tile_pool(name="inp", bufs=5))
    work = ctx.enter_context(tc.tile_pool(name="work", bufs=5))
    psum = ctx.enter_context(tc.tile_pool(name="psum", bufs=8, space="PSUM"))

    w_cat = wpool.tile([S, N_MM * H], BF16, name="w_cat")
    nc.sync.dma_start(w_cat[:], io["w_cat"][:])
    wtiles = [w_cat[:, bass.ts(k, H)] for k in range(N_MM)]

    pending_store = None  # (ch_tile, col_offset)

    def flush_store():
        nonlocal pending_store
        if pending_store is not None:
            ch, off, tcols = pending_store
            nc.scalar.dma_start(
                io["out_packed"][:, 2 * off : 2 * off + 2 * tcols], ch[:]
            )
            pending_store = None

    for off, tcols in tile_plan(b_core):
        n_chunks = tcols // MM_FREE
        t_in = inpool.tile([S, N_OPS * tcols], BF16, name="t_in")
        nc.sync.dma_start(
            t_in[:],
            io["in_packed"][:, N_OPS * off : N_OPS * (off + tcols)],
        )
        # operand views inside the packed tile
        ops = [t_in[:, bass.ts(k, tcols)] for k in range(N_MM)]
        t_lc = t_in[:, bass.ts(N_MM, tcols)]

        # weight-stationary sweep: LS w_k once, accumulate into all banks
        pss = [psum.tile([H, MM_FREE], F32, name="ps") for _ in range(n_chunks)]
        for k in range(N_MM):
            for j in range(n_chunks):
                nc.tensor.matmul(
                    pss[j][:], wtiles[k], ops[k][:, bass.ts(j, MM_FREE)],
                    start=(k == 0), stop=(k == N_MM - 1),
                )


        flush_store()  # previous tile's c|h are ready; don't queue the
        # store issue behind this tile's sigmoids on the Scalar FIFO

        g = work.tile([H, tcols], BF16, name="g")
        for j in range(n_chunks):
            nc.scalar.activation(
                g[:, bass.ts(j, MM_FREE)], pss[j][:],
                mybir.ActivationFunctionType.Sigmoid,
            )

        # c = g * (last_c + g); h = g * c  -- all on DVE, back to back
        tmp = work.tile([H, tcols], BF16, name="tmp")
        nc.vector.tensor_add(tmp[:], g[:], t_lc[:])
        ch = work.tile([H, 2 * tcols], BF16, name="ch")
        nc.vector.tensor_mul(ch[:, 0:tcols], g[:], tmp[:])
        nc.vector.tensor_mul(ch[:, tcols : 2 * tcols], g[:], ch[:, 0:tcols])
        pending_store = (ch, off, tcols)

    flush_store()


def build_model(b_core: int = B_CORE, n_cores: int = N_CORES):
    nc = bacc.Bacc(
        "TRN2",
        target_bir_lowering=False,
        debug=False,
        enable_asserts=False,
        num_devices=n_cores,
    )
    io = {}
    io["in_packed"] = nc.dram_tensor(
        "in_packed", [S, N_OPS * b_core], BF16, kind="ExternalInput"
    ).ap()
    io["w_cat"] = nc.dram_tensor(
        "w_cat", [S, N_MM * H], BF16, kind="ExternalInput"
    ).ap()
    io["out_packed"] = nc.dram_tensor(
        "out_packed", [H, 2 * b_core], BF16, kind="ExternalOutput"
    ).ap()

    with tile.TileContext(nc) as tc, ExitStack() as ctx:
        emit_lstm_tile(ctx, tc, io, b_core)
    nc.compile()
    return nc


_model_cache: dict = {}


def _get_model():
    if "nc" not in _model_cache:
        _model_cache["nc"] = build_model()
    return _model_cache["nc"]


def make_in_maps(inputs: dict, b_core: int = B_CORE, n_cores: int = N_CORES):
    w_cat = np.concatenate(
        [np.asarray(inputs[w], dtype=np.float32).T for w in WEIGHTS], axis=1
    ).astype(NP_BF16)
    big = {k: np.asarray(inputs[k], dtype=np.float32) for k in MM_INPUTS + ("last_c",)}
    in_maps = []
    for c in range(n_cores):
        sl = slice(c * b_core, (c + 1) * b_core)
        m = {
            "in_packed": pack_mm_inputs(
                [big[k][:, sl] for k in MM_INPUTS + ("last_c",)], b_core
            ),
            "w_cat": w_cat,
        }
        in_maps.append(m)
    return in_maps


def run_spmd(inputs: dict, trace: bool = False, **kwargs):
    nc = _get_model()
    in_maps = make_in_maps(inputs)
    res = bass_utils.run_bass_kernel_spmd(
        nc, in_maps, core_ids=list(range(N_CORES)), trace=trace, **kwargs
    )
    cells, hiddens = [], []
    for c in range(N_CORES):
        cell, hidden = unpack_outputs(res.results[c]["out_packed"], B_CORE)
        cells.append(cell)
        hiddens.append(hidden)
    return (
        np.concatenate(cells, axis=1),
        np.concatenate(hiddens, axis=1),
    ), res


def kernel(**inputs):
    outs, _ = run_spmd(inputs, trace=False)
    return outs
